# revision 37
# baseline (speedup 1.0000x reference)
"""MultiHeadAttention kernel for Trainium2, 8-core head-parallel.

Problem: S=2048, B=2, D=1024, 16 heads of d=64 (batch_first=False).
Sharding: tensor-parallel over heads — each of the 8 cores computes 2 heads
(a 128-column slice of the output). No collectives: every core gets the full
(bf16, transposed) activations plus its own weight slice, computes its output
slice, and the host concatenates.

Per-core dataflow (layouts chosen so only V needs an on-device transpose,
done on the PE):
  q^T, k^T  [128=2*64 dout, T] = W_slice @ x^T          (PE, bf16, fp32 psum)
  v^T       [128, T] likewise, then PE-transposed to token-major v' [tok, 65]
            per head with a ones column appended (for softmax denominators)
  scores^T  [j, i] = k_h-tile . q_h-tile                (PE, K=64; both heads
            write ONE [128, 2*512] psum tile: h0 bank0, h1 bank1)
  attn^T    = exp(scores * 1/8) for BOTH heads in a single [128, 1024]
            activation per point (ScalarE is the near-critical engine at
            ~1.04us/point; one instruction halves its fixed overheads)
  pv^T      [65, i] = v'^T . attn^T  — row 64 = softmax denominator.
            PV is software-pipelined ONE j-tile behind the exp: the PE order
            is S(j), PV(j-1), S(j+1), ... so the PE never waits on the
            ScalarE inside a point, stays dense, and the HAM monitor keeps
            it at 2.4 GHz.
  out^T     [64, i] = pv^T[0:64] * (1/pv^T[64])         (den row copied to a
            partition-0 tile, broadcast over 64 partitions by GpSimd
            partition_broadcast, then DVE reciprocal + multiply — no PE or
            PSUM involvement, so the pv psum slot frees at the evacuation)
Projections are split into load units (SWDGE dma) and compute units
(8 matmuls + DVE bias add), deadline-scheduled into the attention point
stream so the PE stream stays dense while the ScalarE paces the kernel.
Host gathers out^T [128, B*S] per core -> [S, B, 1024].
"""

import sys

if "/opt/trn_rl_repo" not in sys.path:
    sys.path.insert(0, "/opt/trn_rl_repo")

import numpy as np
import ml_dtypes

import concourse.bass as bass
import concourse.mybir as mybir
import concourse.tile as tile
from concourse import bacc

BF16 = mybir.dt.bfloat16
FP32 = mybir.dt.float32
FP8 = mybir.dt.float8e4
FP32R = mybir.dt.float32r
NP_BF16 = ml_dtypes.bfloat16

D = 1024
NHEAD = 16
DH = 64
NCORES = 8
HPC = NHEAD // NCORES        # heads per core = 2
DC = HPC * DH                # per-core output dims = 128
KT = D // 128                # contraction tiles = 8
SCALE = 1.0 / float(np.sqrt(DH))


def build_program(S: int, B: int):
    """Build the single-core Bass program (identical across the 8 cores)."""
    assert S % 512 == 0
    T = S * B
    JT = S // 128                    # key tiles per (b, h)
    IC = 512                         # i-chunk width (one psum bank per head)
    NIC = S // IC
    NPT = NIC * JT                   # attention points per batch
    TB = 512                         # token block for projections
    TPB = S // TB
    VSUB = TB // 128
    JTB = S // 128

    nc = bacc.Bacc(
        "TRN2", target_bir_lowering=False, debug=False, num_devices=NCORES
    )
    NTILE = T // TB
    # pre-tiled on host: tile (b*TPB+tb) is one contiguous [128, KT, TB] block
    xq = nc.dram_tensor("xq", [NTILE, 128, KT, TB], BF16, kind="ExternalInput")
    xk = nc.dram_tensor("xk", [NTILE, 128, KT, TB], BF16, kind="ExternalInput")
    xv = nc.dram_tensor("xv", [NTILE, 128, KT, TB], BF16, kind="ExternalInput")
    wq = nc.dram_tensor("wq", [128, KT, DC], BF16, kind="ExternalInput")
    wk = nc.dram_tensor("wk", [128, KT, DC], BF16, kind="ExternalInput")
    wv = nc.dram_tensor("wv", [128, KT, DC], BF16, kind="ExternalInput")
    bqkv = nc.dram_tensor("bqkv", [DC, 3], FP32, kind="ExternalInput")
    ident = nc.dram_tensor("ident", [128, 128], BF16, kind="ExternalInput")
    out = nc.dram_tensor("out", [DC, T], FP32, kind="ExternalOutput")

    with tile.TileContext(nc) as tc:
        with (
            tc.tile_pool(name="const", bufs=1) as constp,
            tc.tile_pool(name="xin", bufs=1) as xinp,
            tc.tile_pool(name="qkv", bufs=1) as qkvp,
            tc.tile_pool(name="attn", bufs=3) as attnp,
            tc.tile_pool(name="vstg", bufs=2) as vstgp,
            tc.tile_pool(name="drain", bufs=2) as drainp,
            tc.tile_pool(name="outp", bufs=2) as outp,
            tc.tile_pool(name="ps", bufs=1, space="PSUM") as psp,
        ):
            wtmp = constp.tile([128, 128], BF16, tag="wtmp")
            nc.vector.memset(wtmp[:], 1.0)
            ident_t = constp.tile([128, 128], BF16, tag="ident")
            nc.sync.dma_start(out=ident_t[:], in_=ident[:, :])
            wq_t = constp.tile([128, KT, DC], BF16, tag="wq")
            wk_t = constp.tile([128, KT, DC], BF16, tag="wk")
            wv_t = constp.tile([128, KT, DC], BF16, tag="wv")
            nc.sync.dma_start(out=wq_t[:], in_=wq[:, :, :])
            nc.sync.dma_start(out=wk_t[:], in_=wk[:, :, :])
            nc.sync.dma_start(out=wv_t[:], in_=wv[:, :, :])
            bqkv_t = constp.tile([DC, 3], FP32, tag="bqkv")

            q_b = []
            k_b = []
            v_b = []
            for b in range(B):
                q_b.append(qkvp.tile([128, S], BF16, tag=f"q{b}", name=f"q{b}"))
                k_b.append(qkvp.tile([128, S], BF16, tag=f"k{b}", name=f"k{b}"))
                v_b.append(
                    qkvp.tile([128, JTB, HPC, DH + 1], BF16, tag=f"v{b}", name=f"v{b}")
                )

            PS_BUFS = {"sc": 2, "pv": 3, "ps": 1}

            # ---- projection building blocks --------------------------------
            xtiles = {}   # (kind, b, tb) -> staged x tile
            vstage = {}   # (b, tb) -> vT staging tile

            def load_x(kind, b, tb, eng=None, halves=False):
                src = {"q": xq, "k": xk, "v": xv}[kind]
                t = xinp.tile(
                    [128, KT, TB], BF16, tag=f"x{kind}", name=f"x{kind}{b}{tb}",
                    bufs=3,
                )
                if halves:
                    HB = TB // 2
                    (eng or nc.gpsimd).dma_start(
                        out=t[:, :, 0:HB], in_=src[b * TPB + tb, :, :, 0:HB]
                    )
                    (eng or nc.gpsimd).dma_start(
                        out=t[:, :, HB:TB], in_=src[b * TPB + tb, :, :, HB:TB]
                    )
                else:
                    (eng or nc.gpsimd).dma_start(
                        out=t[:], in_=src[b * TPB + tb, :, :, :]
                    )
                xtiles[(kind, b, tb)] = t

            def comp_qk(kind, b, tb, tag="ps", trange=None, keep=False):
                w_t = wq_t if kind == "q" else wk_t
                bcol = 0 if kind == "q" else 1
                dst = q_b[b] if kind == "q" else k_b[b]
                if keep:
                    x_t = xtiles[(kind, b, tb)]
                else:
                    x_t = xtiles.pop((kind, b, tb))
                t0, t1 = trange or (0, TB)
                W = t1 - t0
                ps_x = psp.tile([128, TB], FP32, tag=tag, name=f"ps{kind}",
                                bufs=PS_BUFS[tag])
                for kt in range(KT):
                    nc.tensor.matmul(
                        ps_x[:, :W], w_t[:, kt, :], x_t[:, kt, t0:t1],
                        start=(kt == 0), stop=(kt == KT - 1),
                    )
                s0 = tb * TB + t0
                nc.vector.tensor_add(
                    dst[:, s0 : s0 + W],
                    ps_x[:, :W],
                    bqkv_t[:, bcol : bcol + 1].to_broadcast((DC, W)),
                )

            def comp_v_proj(b, tb, tag="ps", trange=None, keep=False):
                if keep:
                    x_t = xtiles[("v", b, tb)]
                else:
                    x_t = xtiles.pop(("v", b, tb))
                t0, t1 = trange or (0, TB)
                W = t1 - t0
                ps_v = psp.tile([128, TB], FP32, tag=tag, name="psv",
                                bufs=PS_BUFS[tag])
                for kt in range(KT):
                    nc.tensor.matmul(
                        ps_v[:, :W], wv_t[:, kt, :], x_t[:, kt, t0:t1],
                        start=(kt == 0), stop=(kt == KT - 1),
                    )
                if trange is None:
                    vT = vstgp.tile([128, TB], BF16, tag="vT", name="vT")
                    vstage[(b, tb)] = vT
                else:
                    vT = vstage.setdefault(
                        (b, tb),
                        vstgp.tile([128, TB], BF16, tag="vT", name="vT"),
                    )
                nc.vector.tensor_add(
                    vT[:, t0:t1], ps_v[:, :W],
                    bqkv_t[:, 2:3].to_broadcast((DC, W)),
                )

            def comp_v_tr(b, tb, sub, tag="ps"):
                vT = vstage[(b, tb)]
                pt = psp.tile([128, 128], BF16, tag=tag, name="pt", bufs=PS_BUFS[tag])
                nc.tensor.transpose(
                    pt[:, :], vT[:, sub * 128 : (sub + 1) * 128], ident_t[:, :]
                )
                vt_idx = tb * VSUB + sub
                nc.vector.tensor_copy(
                    v_b[b][:, vt_idx, :, 0:DH],
                    pt[:, :].rearrange("p (h d) -> p h d", h=HPC),
                )
                if sub == VSUB - 1:
                    del vstage[(b, tb)]

            # ---- attention -------------------------------------------------
            pend = [None]            # (b, ic, jt, at_tile, pv_pair, is_last)
            cur_pv = [None]          # current chunk's [pv_h0, pv_h1]
            pending_final = []

            def emit_pv(b, jt, at, pvs, is_last):
                for hh in range(HPC):
                    nc.tensor.matmul(
                        pvs[hh][0 : DH + 1, :],
                        v_b[b][:, jt, hh, :],
                        at[:, hh * IC : (hh + 1) * IC],
                        start=(jt == 0), stop=is_last,
                    )

            def emit_evac(b, ic, pvs):
                # Evacuate [65, IC] pv (row 0 = denominator, rows 1:65 = PV)
                # to SBUF in one DVE copy so the psum slot frees immediately
                # (pv's only accessors are the PV matmuls and this copy);
                # broadcast the denominator row — partition 0, where AP base
                # and physical partition agree — over 64 partitions on the
                # idle GpSimd engine. The reciprocal / multiply / store tail
                # is deferred via pending_final so the in-order DVE queue
                # isn't blocked waiting on the broadcast.
                for hh in range(HPC):
                    pv = pvs[hh]
                    pvsb = outp.tile([DH + 1, IC], FP32, tag="pvsb", name="pvsb")
                    nc.vector.tensor_copy(pvsb[:, :], pv[0 : DH + 1, :])
                    # DVE reciprocal is ~6.4 cycles/elem (multi-pass): reshape
                    # the [1, IC] den row to [128, IC/128] via DMA so the
                    # reciprocal is 4 elems/partition, then broadcast the
                    # reciprocal row over 64 partitions on the idle GpSimd.
                    rsh = drainp.tile([128, IC // 128], FP32, tag="rsh", name="rsh")
                    nc.sync.dma_start(out=rsh[:], in_=pvsb[DH : DH + 1, :])
                    rec = drainp.tile([128, IC // 128], FP32, tag="rec", name="rec")
                    nc.vector.reciprocal(rec[:], rsh[:])
                    rrow = drainp.tile([1, IC], FP32, tag="rrow", name="rrow")
                    nc.sync.dma_start(out=rrow[:], in_=rec[:])
                    rec_b = drainp.tile([DH, IC], FP32, tag="recb", name="recb")
                    nc.gpsimd.partition_broadcast(rec_b[:, :], rrow[0:1, :])

                    def finalize(b=b, ic=ic, hh=hh, pvsb=pvsb, rec_b=rec_b):
                        nc.vector.tensor_mul(
                            pvsb[0:DH, :], pvsb[0:DH, :], rec_b[:, :]
                        )
                        nc.sync.dma_start(
                            out=out[
                                hh * DH : (hh + 1) * DH,
                                b * S + ic * IC : b * S + (ic + 1) * IC,
                            ],
                            in_=pvsb[0:DH, :],
                        )

                    pending_final.append(finalize)

            units = []               # (point, order, fn)

            def make_inject(units):
                units = sorted(units, key=lambda u: (u[0], u[1]))
                ui = [0]

                def inject(point):
                    while ui[0] < len(units) and units[ui[0]][0] <= point:
                        units[ui[0]][2]()
                        ui[0] += 1

                return inject, units

            def emit_point(b, ic, jt, pt, inject):
                inject(pt)
                if jt == 3 and pending_final:
                    for fn in pending_final:
                        fn()
                    pending_final.clear()
                sc = psp.tile([128, 2 * IC], FP32, tag="sc", name="sc", bufs=2)
                at = attnp.tile([128, 2 * IC], BF16, tag="at", name="at")
                i0 = ic * IC
                for hh in range(HPC):
                    p0 = hh * DH
                    nc.tensor.matmul(
                        sc[:, hh * IC : (hh + 1) * IC],
                        k_b[b][p0 : p0 + DH, jt * 128 : (jt + 1) * 128],
                        q_b[b][p0 : p0 + DH, i0 : i0 + IC],
                        start=True, stop=True,
                    )
                if pend[0] is not None:
                    pb, pic, pjt, pat, ppvs, plast = pend[0]
                    emit_pv(pb, pjt, pat, ppvs, plast)
                    if plast:
                        emit_evac(pb, pic, ppvs)
                nc.scalar.activation(
                    out=at[:, :], in_=sc[:, :],
                    func=mybir.ActivationFunctionType.Exp,
                    scale=SCALE,
                )
                if jt == 0:
                    cur_pv[0] = [
                        psp.tile([128, IC], FP32, tag="pv", name=f"pv{hh}", bufs=3)
                        for hh in range(HPC)
                    ]
                pend[0] = (b, ic, jt, at, cur_pv[0], jt == JT - 1)

            # ---- deadline schedule for projection units --------------------
            prologue = []

            def add_unit(ptx, order, fn):
                if ptx < 0:
                    prologue.append((ptx, order, fn))
                else:
                    units.append((ptx, order, fn))

            for b in range(B):
                base = b * NPT
                for tb in range(TPB):
                    kdl = base + 4 * tb       # k(tb) used by scores at jt=4tb
                    vdl = base + 4 * tb + 1   # v(tb) used by PV one point later
                    qdl = base + 16 * tb      # q(tb) used by chunk tb
                    if b == 0 and tb == 0:
                        continue               # handled in prologue
                    add_unit(kdl - 10, 0, lambda b=b, tb=tb: load_x("k", b, tb))
                    add_unit(kdl - 3, 1, lambda b=b, tb=tb: comp_qk("k", b, tb))
                    add_unit(vdl - 12, 0, lambda b=b, tb=tb: load_x("v", b, tb))
                    add_unit(vdl - 7, 1, lambda b=b, tb=tb: comp_v_proj(b, tb))
                    for sub in range(VSUB):
                        add_unit(
                            vdl - 5 + sub, 1,
                            lambda b=b, tb=tb, sub=sub: comp_v_tr(b, tb, sub),
                        )
                    add_unit(qdl - 10, 0, lambda b=b, tb=tb: load_x("q", b, tb))
                    add_unit(qdl - 3, 1, lambda b=b, tb=tb: comp_qk("q", b, tb))
                if b == 1:
                    add_unit(base - 14, 0, lambda b=b: nc.vector.memset(
                        v_b[b][:, :, :, DH : DH + 1], 1.0))

            # ---- prologue --------------------------------------------------
            # dense dummy-matmul burst on the identity (the first DMA to
            # land): fires the PE HAM monitor toward full clock while the
            # weight / x-tile DMAs stream in behind it
            warm = psp.tile([128, 2 * IC], FP32, tag="sc", name="warm", bufs=2)
            for _ in range(40):
                nc.tensor.matmul(
                    warm[:, :128], wtmp[:, :], wtmp[:, :],
                    start=True, stop=True,
                )
            load_x("q", 0, 0)
            load_x("k", 0, 0, halves=True)
            load_x("v", 0, 0, halves=True)
            nc.sync.dma_start(out=bqkv_t[:], in_=bqkv[:, :])
            nc.vector.memset(v_b[0][:, :, :, DH : DH + 1], 1.0)
            comp_qk("q", 0, 0, tag="sc")
            HB = TB // 2
            comp_qk("k", 0, 0, tag="sc", trange=(0, HB), keep=True)
            comp_v_proj(0, 0, tag="sc", trange=(0, HB), keep=True)
            comp_v_tr(0, 0, 0, tag="sc")
            comp_v_tr(0, 0, 1, tag="sc")
            comp_qk("k", 0, 0, tag="sc", trange=(HB, TB))
            comp_v_proj(0, 0, tag="sc", trange=(HB, TB))
            comp_v_tr(0, 0, 2, tag="sc")
            comp_v_tr(0, 0, 3, tag="sc")
            for _, _, fn in sorted(prologue, key=lambda u: (u[0], u[1])):
                fn()

            # ---- main loop -------------------------------------------------
            inject, _ = make_inject(units)
            for b in range(B):
                for ic in range(NIC):
                    for jt in range(JT):
                        pt = b * NPT + ic * JT + jt
                        emit_point(b, ic, jt, pt, inject)
            inject(10 ** 9)
            # flush the trailing PV / evac / finalize
            pb, pic, pjt, pat, ppvs, plast = pend[0]
            emit_pv(pb, pjt, pat, ppvs, plast)
            emit_evac(pb, pic, ppvs)
            pend[0] = None
            for fn in pending_final:
                fn()
            pending_final.clear()

    nc.finalize()
    return nc


_PROGRAM_CACHE = {}


def _get_program(S, B):
    key = (S, B)
    if key not in _PROGRAM_CACHE:
        _PROGRAM_CACHE[key] = build_program(S, B)
    return _PROGRAM_CACHE[key]


def make_in_maps(query, key, value, Wq, bq, Wk, bk, Wv, bv):
    S, B, D_ = query.shape
    assert D_ == D
    T = S * B

    TB = 512 if S % 512 == 0 else S
    NTILE = T // TB

    def xt(a):
        # [S, B, D] -> transposed [D, B*S] -> pre-tiled [NTILE, 128, KT, TB]
        # with dim (2f+r)*64+p at partition r*64+p, free index f: the r=0/r=1
        # projection matmuls then use disjoint PE row groups (LDW overlap +
        # concurrent streams). bf16 so each tile is one contiguous 1MB DMA.
        aT = np.asarray(a, np.float32).transpose(2, 1, 0).reshape(D_, T)
        a4 = aT.reshape(KT, 2, 64, NTILE, TB).transpose(3, 1, 2, 0, 4)
        return np.ascontiguousarray(a4.reshape(NTILE, 128, KT, TB)).astype(NP_BF16)

    xqh, xkh, xvh = xt(query), xt(key), xt(value)
    identh = np.eye(128, dtype=NP_BF16)
    in_maps = []
    for c in range(NCORES):
        rows = slice(c * DC, (c + 1) * DC)

        def wt(W):
            # [D, DC] col-slice, pre-arranged to [128, KT, DC] matching xt()
            wT = np.asarray(W)[rows, :].T.reshape(KT, 2, 64, DC)
            return np.ascontiguousarray(
                wT.transpose(1, 2, 0, 3).reshape(128, KT, DC)
            ).astype(NP_BF16)
        in_maps.append(
            {
                "xq": xqh, "xk": xkh, "xv": xvh,
                "wq": wt(Wq), "wk": wt(Wk), "wv": wt(Wv),
                "bqkv": np.ascontiguousarray(
                    np.stack(
                        [np.asarray(bq)[rows], np.asarray(bk)[rows], np.asarray(bv)[rows]],
                        axis=1,
                    )
                ).astype(np.float32),
                "ident": identh,
            }
        )
    return in_maps


def gather_output(results, S, B):
    full = np.empty((S, B, D), np.float32)
    for c in range(NCORES):
        o = np.asarray(results[c]["out"], np.float32)  # [DC, B*S]
        full[:, :, c * DC : (c + 1) * DC] = o.reshape(DC, B, S).transpose(2, 1, 0)
    return full


def kernel(query, key, value, Wq, bq, Wk, bk, Wv, bv):
    from concourse.bass_utils import run_bass_kernel_spmd

    S, B, _ = query.shape
    nc = _get_program(S, B)
    in_maps = make_in_maps(query, key, value, Wq, bq, Wk, bk, Wv, bv)
    res = run_bass_kernel_spmd(nc, in_maps, list(range(NCORES)))
    return gather_output(res.results, S, B)


# revision 38
# speedup vs baseline: 1.0157x; 1.0157x over previous
"""MultiHeadAttention kernel for Trainium2, 8-core head-parallel.

Problem: S=2048, B=2, D=1024, 16 heads of d=64 (batch_first=False).
Sharding: tensor-parallel over heads — each of the 8 cores computes 2 heads
(a 128-column slice of the output). No collectives: every core gets the full
(bf16, transposed) activations plus its own weight slice, computes its output
slice, and the host concatenates.

Per-core dataflow (layouts chosen so only V needs an on-device transpose,
done on the PE):
  q^T, k^T  [128=2*64 dout, T] = W_slice @ x^T          (PE, bf16, fp32 psum)
  v^T       [128, T] likewise, then PE-transposed to token-major v' [tok, 65]
            per head with a ones column appended (for softmax denominators)
  scores^T  [j, i] = k_h-tile . q_h-tile                (PE, K=64; both heads
            write ONE [128, 2*512] psum tile: h0 bank0, h1 bank1)
  attn^T    = exp(scores * 1/8) for BOTH heads in a single [128, 1024]
            activation per point (ScalarE is the near-critical engine at
            ~1.04us/point; one instruction halves its fixed overheads)
  pv^T      [65, i] = v'^T . attn^T  — row 64 = softmax denominator.
            PV is software-pipelined ONE j-tile behind the exp: the PE order
            is S(j), PV(j-1), S(j+1), ... so the PE never waits on the
            ScalarE inside a point, stays dense, and the HAM monitor keeps
            it at 2.4 GHz.
  out^T     [64, i] = pv^T[0:64] * (1/pv^T[64])         (den row copied to a
            partition-0 tile, broadcast over 64 partitions by GpSimd
            partition_broadcast, then DVE reciprocal + multiply — no PE or
            PSUM involvement, so the pv psum slot frees at the evacuation)
Projections are split into load units (SWDGE dma) and compute units
(8 matmuls + DVE bias add), deadline-scheduled into the attention point
stream so the PE stream stays dense while the ScalarE paces the kernel.
Host gathers out^T [128, B*S] per core -> [S, B, 1024].
"""

import sys

if "/opt/trn_rl_repo" not in sys.path:
    sys.path.insert(0, "/opt/trn_rl_repo")

import numpy as np
import ml_dtypes

import concourse.bass as bass
import concourse.mybir as mybir
import concourse.tile as tile
from concourse import bacc

BF16 = mybir.dt.bfloat16
FP32 = mybir.dt.float32
FP8 = mybir.dt.float8e4
FP32R = mybir.dt.float32r
NP_BF16 = ml_dtypes.bfloat16

D = 1024
NHEAD = 16
DH = 64
NCORES = 8
HPC = NHEAD // NCORES        # heads per core = 2
DC = HPC * DH                # per-core output dims = 128
KT = D // 128                # contraction tiles = 8
SCALE = 1.0 / float(np.sqrt(DH))


def build_program(S: int, B: int):
    """Build the single-core Bass program (identical across the 8 cores)."""
    assert S % 512 == 0
    T = S * B
    JT = S // 128                    # key tiles per (b, h)
    IC = 512                         # i-chunk width (one psum bank per head)
    NIC = S // IC
    NPT = NIC * JT                   # attention points per batch
    TB = 512                         # token block for projections
    TPB = S // TB
    VSUB = TB // 128
    JTB = S // 128

    nc = bacc.Bacc(
        "TRN2", target_bir_lowering=False, debug=False, num_devices=NCORES
    )
    NTILE = T // TB
    # pre-tiled on host: tile (b*TPB+tb) is one contiguous [128, KT, TB] block
    xq = nc.dram_tensor("xq", [NTILE, 128, KT, TB], BF16, kind="ExternalInput")
    xk = nc.dram_tensor("xk", [NTILE, 128, KT, TB], BF16, kind="ExternalInput")
    xv = nc.dram_tensor("xv", [NTILE, 128, KT, TB], BF16, kind="ExternalInput")
    wq = nc.dram_tensor("wq", [128, KT, DC], BF16, kind="ExternalInput")
    wk = nc.dram_tensor("wk", [128, KT, DC], BF16, kind="ExternalInput")
    wv = nc.dram_tensor("wv", [128, KT, DC], BF16, kind="ExternalInput")
    bqkv = nc.dram_tensor("bqkv", [DC, 3], FP32, kind="ExternalInput")
    ident = nc.dram_tensor("ident", [128, 128], BF16, kind="ExternalInput")
    out = nc.dram_tensor("out", [DC, T], FP32, kind="ExternalOutput")

    with tile.TileContext(nc) as tc:
        with (
            tc.tile_pool(name="const", bufs=1) as constp,
            tc.tile_pool(name="xin", bufs=1) as xinp,
            tc.tile_pool(name="qkv", bufs=1) as qkvp,
            tc.tile_pool(name="attn", bufs=4) as attnp,
            tc.tile_pool(name="vstg", bufs=2) as vstgp,
            tc.tile_pool(name="drain", bufs=2) as drainp,
            tc.tile_pool(name="outp", bufs=2) as outp,
            tc.tile_pool(name="ps", bufs=1, space="PSUM") as psp,
        ):
            wtmp = constp.tile([128, 128], BF16, tag="wtmp")
            nc.vector.memset(wtmp[:], 1.0)
            ident_t = constp.tile([128, 128], BF16, tag="ident")
            nc.sync.dma_start(out=ident_t[:], in_=ident[:, :])
            wq_t = constp.tile([128, KT, DC], BF16, tag="wq")
            wk_t = constp.tile([128, KT, DC], BF16, tag="wk")
            wv_t = constp.tile([128, KT, DC], BF16, tag="wv")
            nc.sync.dma_start(out=wq_t[:], in_=wq[:, :, :])
            nc.sync.dma_start(out=wk_t[:], in_=wk[:, :, :])
            nc.sync.dma_start(out=wv_t[:], in_=wv[:, :, :])
            bqkv_t = constp.tile([DC, 3], FP32, tag="bqkv")

            q_b = []
            k_b = []
            v_b = []
            for b in range(B):
                q_b.append(qkvp.tile([128, S], BF16, tag=f"q{b}", name=f"q{b}"))
                k_b.append(qkvp.tile([128, S], BF16, tag=f"k{b}", name=f"k{b}"))
                v_b.append(
                    qkvp.tile([128, JTB, HPC, DH + 1], BF16, tag=f"v{b}", name=f"v{b}")
                )

            PS_BUFS = {"sc": 2, "pv": 3, "ps": 1}

            # ---- projection building blocks --------------------------------
            xtiles = {}   # (kind, b, tb) -> staged x tile
            vstage = {}   # (b, tb) -> vT staging tile

            def load_x(kind, b, tb, eng=None, halves=False):
                src = {"q": xq, "k": xk, "v": xv}[kind]
                t = xinp.tile(
                    [128, KT, TB], BF16, tag=f"x{kind}", name=f"x{kind}{b}{tb}",
                    bufs=3,
                )
                if halves:
                    HB = TB // 2
                    (eng or nc.gpsimd).dma_start(
                        out=t[:, :, 0:HB], in_=src[b * TPB + tb, :, :, 0:HB]
                    )
                    (eng or nc.gpsimd).dma_start(
                        out=t[:, :, HB:TB], in_=src[b * TPB + tb, :, :, HB:TB]
                    )
                else:
                    (eng or nc.gpsimd).dma_start(
                        out=t[:], in_=src[b * TPB + tb, :, :, :]
                    )
                xtiles[(kind, b, tb)] = t

            def comp_qk(kind, b, tb, tag="ps", trange=None, keep=False):
                w_t = wq_t if kind == "q" else wk_t
                bcol = 0 if kind == "q" else 1
                dst = q_b[b] if kind == "q" else k_b[b]
                if keep:
                    x_t = xtiles[(kind, b, tb)]
                else:
                    x_t = xtiles.pop((kind, b, tb))
                t0, t1 = trange or (0, TB)
                W = t1 - t0
                ps_x = psp.tile([128, TB], FP32, tag=tag, name=f"ps{kind}",
                                bufs=PS_BUFS[tag])
                for kt in range(KT):
                    nc.tensor.matmul(
                        ps_x[:, :W], w_t[:, kt, :], x_t[:, kt, t0:t1],
                        start=(kt == 0), stop=(kt == KT - 1),
                    )
                s0 = tb * TB + t0
                nc.vector.tensor_add(
                    dst[:, s0 : s0 + W],
                    ps_x[:, :W],
                    bqkv_t[:, bcol : bcol + 1].to_broadcast((DC, W)),
                )

            def comp_v_proj(b, tb, tag="ps", trange=None, keep=False):
                if keep:
                    x_t = xtiles[("v", b, tb)]
                else:
                    x_t = xtiles.pop(("v", b, tb))
                t0, t1 = trange or (0, TB)
                W = t1 - t0
                ps_v = psp.tile([128, TB], FP32, tag=tag, name="psv",
                                bufs=PS_BUFS[tag])
                for kt in range(KT):
                    nc.tensor.matmul(
                        ps_v[:, :W], wv_t[:, kt, :], x_t[:, kt, t0:t1],
                        start=(kt == 0), stop=(kt == KT - 1),
                    )
                if trange is None:
                    vT = vstgp.tile([128, TB], BF16, tag="vT", name="vT")
                    vstage[(b, tb)] = vT
                else:
                    vT = vstage.setdefault(
                        (b, tb),
                        vstgp.tile([128, TB], BF16, tag="vT", name="vT"),
                    )
                nc.vector.tensor_add(
                    vT[:, t0:t1], ps_v[:, :W],
                    bqkv_t[:, 2:3].to_broadcast((DC, W)),
                )

            def comp_v_tr(b, tb, sub, tag="ps"):
                vT = vstage[(b, tb)]
                pt = psp.tile([128, 128], BF16, tag=tag, name="pt", bufs=PS_BUFS[tag])
                nc.tensor.transpose(
                    pt[:, :], vT[:, sub * 128 : (sub + 1) * 128], ident_t[:, :]
                )
                vt_idx = tb * VSUB + sub
                nc.vector.tensor_copy(
                    v_b[b][:, vt_idx, :, 0:DH],
                    pt[:, :].rearrange("p (h d) -> p h d", h=HPC),
                )
                if sub == VSUB - 1:
                    del vstage[(b, tb)]

            # ---- attention -------------------------------------------------
            pend = [None]            # (b, ic, jt, at_tile, pv_pair, is_last)
            cur_pv = [None]          # current chunk's [pv_h0, pv_h1]
            pending_final = []

            def emit_pv(b, jt, at, pvs, is_last):
                for hh in range(HPC):
                    nc.tensor.matmul(
                        pvs[hh][0 : DH + 1, :],
                        v_b[b][:, jt, hh, :],
                        at[:, hh * IC : (hh + 1) * IC],
                        start=(jt == 0), stop=is_last,
                    )

            def emit_evac(b, ic, pvs):
                # Evacuate [65, IC] pv (row 0 = denominator, rows 1:65 = PV)
                # to SBUF in one DVE copy so the psum slot frees immediately
                # (pv's only accessors are the PV matmuls and this copy);
                # broadcast the denominator row — partition 0, where AP base
                # and physical partition agree — over 64 partitions on the
                # idle GpSimd engine. The reciprocal / multiply / store tail
                # is deferred via pending_final so the in-order DVE queue
                # isn't blocked waiting on the broadcast.
                for hh in range(HPC):
                    pv = pvs[hh]
                    pvsb = outp.tile([DH + 1, IC], FP32, tag="pvsb", name="pvsb")
                    nc.vector.tensor_copy(pvsb[:, :], pv[0 : DH + 1, :])
                    # DVE reciprocal is ~6.4 cycles/elem (multi-pass): reshape
                    # the [1, IC] den row to [128, IC/128] via DMA so the
                    # reciprocal is 4 elems/partition, then broadcast the
                    # reciprocal row over 64 partitions on the idle GpSimd.
                    rsh = drainp.tile([128, IC // 128], FP32, tag="rsh", name="rsh")
                    nc.sync.dma_start(out=rsh[:], in_=pvsb[DH : DH + 1, :])
                    rec = drainp.tile([128, IC // 128], FP32, tag="rec", name="rec")
                    nc.vector.reciprocal(rec[:], rsh[:])
                    rrow = drainp.tile([1, IC], FP32, tag="rrow", name="rrow")
                    nc.sync.dma_start(out=rrow[:], in_=rec[:])
                    rec_b = drainp.tile([DH, IC], FP32, tag="recb", name="recb")
                    nc.gpsimd.partition_broadcast(rec_b[:, :], rrow[0:1, :])

                    def finalize(b=b, ic=ic, hh=hh, pvsb=pvsb, rec_b=rec_b):
                        nc.vector.tensor_mul(
                            pvsb[0:DH, :], pvsb[0:DH, :], rec_b[:, :]
                        )
                        nc.sync.dma_start(
                            out=out[
                                hh * DH : (hh + 1) * DH,
                                b * S + ic * IC : b * S + (ic + 1) * IC,
                            ],
                            in_=pvsb[0:DH, :],
                        )

                    pending_final.append(finalize)

            units = []               # (point, order, fn)

            def make_inject(units):
                units = sorted(units, key=lambda u: (u[0], u[1]))
                ui = [0]

                def inject(point):
                    while ui[0] < len(units) and units[ui[0]][0] <= point:
                        units[ui[0]][2]()
                        ui[0] += 1

                return inject, units

            def emit_point(b, ic, jt, pt, inject):
                inject(pt)
                if jt == 3 and pending_final:
                    for fn in pending_final:
                        fn()
                    pending_final.clear()
                sc = psp.tile([128, 2 * IC], FP32, tag="sc", name="sc", bufs=2)
                at = attnp.tile([128, 2 * IC], BF16, tag="at", name="at")
                i0 = ic * IC
                for hh in range(HPC):
                    p0 = hh * DH
                    nc.tensor.matmul(
                        sc[:, hh * IC : (hh + 1) * IC],
                        k_b[b][p0 : p0 + DH, jt * 128 : (jt + 1) * 128],
                        q_b[b][p0 : p0 + DH, i0 : i0 + IC],
                        start=True, stop=True,
                    )
                if pend[0] is not None:
                    pb, pic, pjt, pat, ppvs, plast = pend[0]
                    emit_pv(pb, pjt, pat, ppvs, plast)
                    if plast:
                        emit_evac(pb, pic, ppvs)
                nc.scalar.activation(
                    out=at[:, :], in_=sc[:, :],
                    func=mybir.ActivationFunctionType.Exp,
                    scale=SCALE,
                )
                if jt == 0:
                    cur_pv[0] = [
                        psp.tile([128, IC], FP32, tag="pv", name=f"pv{hh}", bufs=3)
                        for hh in range(HPC)
                    ]
                pend[0] = (b, ic, jt, at, cur_pv[0], jt == JT - 1)

            # ---- deadline schedule for projection units --------------------
            prologue = []

            def add_unit(ptx, order, fn):
                if ptx < 0:
                    prologue.append((ptx, order, fn))
                else:
                    units.append((ptx, order, fn))

            for b in range(B):
                base = b * NPT
                for tb in range(TPB):
                    kdl = base + 4 * tb       # k(tb) used by scores at jt=4tb
                    vdl = base + 4 * tb + 1   # v(tb) used by PV one point later
                    qdl = base + 16 * tb      # q(tb) used by chunk tb
                    if b == 0 and tb == 0:
                        continue               # handled in prologue
                    add_unit(kdl - 10, 0, lambda b=b, tb=tb: load_x("k", b, tb))
                    add_unit(kdl - 3, 1, lambda b=b, tb=tb: comp_qk("k", b, tb))
                    add_unit(vdl - 12, 0, lambda b=b, tb=tb: load_x("v", b, tb))
                    add_unit(vdl - (9 if b else 7), 1, lambda b=b, tb=tb: comp_v_proj(b, tb))
                    for sub in range(VSUB):
                        add_unit(
                            vdl - (7 if b else 5) + sub, 1,
                            lambda b=b, tb=tb, sub=sub: comp_v_tr(b, tb, sub),
                        )
                    add_unit(qdl - 10, 0, lambda b=b, tb=tb: load_x("q", b, tb))
                    add_unit(qdl - 3, 1, lambda b=b, tb=tb: comp_qk("q", b, tb))
                if b == 1:
                    add_unit(base - 14, 0, lambda b=b: nc.vector.memset(
                        v_b[b][:, :, :, DH : DH + 1], 1.0))

            # ---- prologue --------------------------------------------------
            # dense dummy-matmul burst on the identity (the first DMA to
            # land): fires the PE HAM monitor toward full clock while the
            # weight / x-tile DMAs stream in behind it
            warm = psp.tile([128, 2 * IC], FP32, tag="sc", name="warm", bufs=2)
            for _ in range(40):
                nc.tensor.matmul(
                    warm[:, :128], wtmp[:, :], wtmp[:, :],
                    start=True, stop=True,
                )
            load_x("q", 0, 0)
            load_x("k", 0, 0, halves=True)
            load_x("v", 0, 0, halves=True)
            nc.sync.dma_start(out=bqkv_t[:], in_=bqkv[:, :])
            nc.vector.memset(v_b[0][:, :, :, DH : DH + 1], 1.0)
            comp_qk("q", 0, 0, tag="sc")
            HB = TB // 2
            comp_qk("k", 0, 0, tag="sc", trange=(0, HB), keep=True)
            comp_v_proj(0, 0, tag="sc", trange=(0, HB), keep=True)
            comp_v_tr(0, 0, 0, tag="sc")
            comp_v_tr(0, 0, 1, tag="sc")
            comp_qk("k", 0, 0, tag="sc", trange=(HB, TB))
            comp_v_proj(0, 0, tag="sc", trange=(HB, TB))
            comp_v_tr(0, 0, 2, tag="sc")
            comp_v_tr(0, 0, 3, tag="sc")
            for _, _, fn in sorted(prologue, key=lambda u: (u[0], u[1])):
                fn()

            # ---- main loop -------------------------------------------------
            inject, _ = make_inject(units)
            for b in range(B):
                for ic in range(NIC):
                    for jt in range(JT):
                        pt = b * NPT + ic * JT + jt
                        emit_point(b, ic, jt, pt, inject)
            inject(10 ** 9)
            # flush the trailing PV / evac / finalize
            pb, pic, pjt, pat, ppvs, plast = pend[0]
            emit_pv(pb, pjt, pat, ppvs, plast)
            emit_evac(pb, pic, ppvs)
            pend[0] = None
            for fn in pending_final:
                fn()
            pending_final.clear()

    nc.finalize()
    return nc


_PROGRAM_CACHE = {}


def _get_program(S, B):
    key = (S, B)
    if key not in _PROGRAM_CACHE:
        _PROGRAM_CACHE[key] = build_program(S, B)
    return _PROGRAM_CACHE[key]


def make_in_maps(query, key, value, Wq, bq, Wk, bk, Wv, bv):
    S, B, D_ = query.shape
    assert D_ == D
    T = S * B

    TB = 512 if S % 512 == 0 else S
    NTILE = T // TB

    def xt(a):
        # [S, B, D] -> transposed [D, B*S] -> pre-tiled [NTILE, 128, KT, TB]
        # with dim (2f+r)*64+p at partition r*64+p, free index f: the r=0/r=1
        # projection matmuls then use disjoint PE row groups (LDW overlap +
        # concurrent streams). bf16 so each tile is one contiguous 1MB DMA.
        aT = np.asarray(a, np.float32).transpose(2, 1, 0).reshape(D_, T)
        a4 = aT.reshape(KT, 2, 64, NTILE, TB).transpose(3, 1, 2, 0, 4)
        return np.ascontiguousarray(a4.reshape(NTILE, 128, KT, TB)).astype(NP_BF16)

    xqh, xkh, xvh = xt(query), xt(key), xt(value)
    identh = np.eye(128, dtype=NP_BF16)
    in_maps = []
    for c in range(NCORES):
        rows = slice(c * DC, (c + 1) * DC)

        def wt(W):
            # [D, DC] col-slice, pre-arranged to [128, KT, DC] matching xt()
            wT = np.asarray(W)[rows, :].T.reshape(KT, 2, 64, DC)
            return np.ascontiguousarray(
                wT.transpose(1, 2, 0, 3).reshape(128, KT, DC)
            ).astype(NP_BF16)
        in_maps.append(
            {
                "xq": xqh, "xk": xkh, "xv": xvh,
                "wq": wt(Wq), "wk": wt(Wk), "wv": wt(Wv),
                "bqkv": np.ascontiguousarray(
                    np.stack(
                        [np.asarray(bq)[rows], np.asarray(bk)[rows], np.asarray(bv)[rows]],
                        axis=1,
                    )
                ).astype(np.float32),
                "ident": identh,
            }
        )
    return in_maps


def gather_output(results, S, B):
    full = np.empty((S, B, D), np.float32)
    for c in range(NCORES):
        o = np.asarray(results[c]["out"], np.float32)  # [DC, B*S]
        full[:, :, c * DC : (c + 1) * DC] = o.reshape(DC, B, S).transpose(2, 1, 0)
    return full


def kernel(query, key, value, Wq, bq, Wk, bk, Wv, bv):
    from concourse.bass_utils import run_bass_kernel_spmd

    S, B, _ = query.shape
    nc = _get_program(S, B)
    in_maps = make_in_maps(query, key, value, Wq, bq, Wk, bk, Wv, bv)
    res = run_bass_kernel_spmd(nc, in_maps, list(range(NCORES)))
    return gather_output(res.results, S, B)
